# revision 54
# baseline (speedup 1.0000x reference)
"""CopyGenerator on 8 TRN2 NeuronCores.

Strategy: tensor-parallel split of the 50257-wide generator vocab across the
8 cores (6284 padded columns each).  Each core:
  - holds its W_gen shard (bf16, pre-transposed on host) resident in SBUF,
  - computes logits = hidden @ W_shard.T with bf16 matmuls (fp32 PSUM accum),
  - applies exp on the Scalar engine (accum_out gives the row partial sums),
  - AllReduce(add)s the softmax partial denominators across cores per 128-row
    tile ([128] f32 — tiny, overlapped with the next tile's matmuls),
  - scales exp by (1 - p_copy)/denom and writes its output shard (bf16,
    converted to f32 on the host),
  - computes the (tiny) copy-attention path redundantly.

Scheduling design (v2): nothing the PE needs ever waits on a collective.
  - pre-collective denominator chain runs entirely on the Scalar engine
    (exp accum -> Identity-accum with bias=-mask/13 -> DMA to DRAM),
  - the collective is AllReduce(add) so the post chain is just one DMA +
    reciprocal + mul on Vector,
  - a dummy AllReduce at kernel start absorbs the CC rendezvous barrier,
  - copy path gets its own PSUM buffers so it never serializes the PE queue,
  - ht is loaded tile-major so the first matmul starts ~1us in.

PAD column and vocab-padding columns are handled by zeroing those W rows on
the host (=> logit 0, exp 1) and subtracting the per-core masked-column count
from the partial denominator; the host zeroes the PAD output column.

kernel(**inputs) takes the full unsharded inputs and returns the full
[2048, 50321] float32 output.
"""

import os
import sys

for _p in ("/opt/trn_rl_repo", "/opt/trn_rl_repo/concourse"):
    if _p not in sys.path:
        sys.path.insert(0, _p)

from contextlib import ExitStack

import ml_dtypes
import numpy as np

import concourse.bass as bass
import concourse.mybir as mybir
import concourse.tile as tile
from concourse import bacc
from concourse.bass_utils import run_bass_kernel_spmd

# ---- problem constants (hardcoded per the self-contained-kernel contract) ----
N, D = 2048, 1024                 # tlen*batch rows, hidden dim
TLEN, BATCH, SLEN, CVOCAB = 64, 32, 128, 64
VOCAB = 50257
PAD_IDX = 0
NCORES = 8
VS = 6284                         # per-core padded vocab shard width
VPAD = VS * NCORES                # 50272
DT = D // 128                     # 8 contraction tiles
NT = N // 128                     # 16 row tiles
CH_W = [512] * 12 + [140]         # vocab chunk widths inside a shard
CH_O = [sum(CH_W[:i]) for i in range(len(CH_W))]
NCH = len(CH_W)

BF16 = ml_dtypes.bfloat16
F32 = mybir.dt.float32
BF16_T = mybir.dt.bfloat16
FP8_T = mybir.dt.float8e4

LAST_RESULTS = None               # BassKernelResults of the most recent run
_NC_CACHE = {}
DEBUG_TAPS = os.environ.get("KERNEL_DEBUG_TAPS", "0") == "1"
USE_FP8 = os.environ.get("KERNEL_FP8", "1") == "1"
FP8 = ml_dtypes.float8_e4m3
W_SCALE = 64.0                    # fp8 weight pre-scale, undone in the Exp


def _build(bc_val: float, use_bgen: bool):
    nc = bacc.Bacc("TRN2", target_bir_lowering=False, debug=False,
                   num_devices=NCORES)

    MAIN_T = FP8_T if USE_FP8 else BF16_T
    wt = nc.dram_tensor("wt", [128, DT * VS], MAIN_T, kind="ExternalInput").ap()
    ht = nc.dram_tensor("ht", [128, NT * 1024], MAIN_T,
                        kind="ExternalInput").ap()
    if USE_FP8:
        htz = nc.dram_tensor("htz", [128, NT * 1024], BF16_T,
                             kind="ExternalInput").ap()
    attn_r = nc.dram_tensor("attn_r", [128, BATCH * TLEN], BF16_T,
                            kind="ExternalInput").ap()
    smap = nc.dram_tensor("smap", [128, BATCH * CVOCAB], BF16_T,
                          kind="ExternalInput").ap()
    wc = nc.dram_tensor("wc", [128, DT], BF16_T, kind="ExternalInput").ap()
    mneg13 = nc.dram_tensor("mneg13", [1, 1], F32, kind="ExternalInput").ap()
    if use_bgen:
        bg = nc.dram_tensor("bg", [1, VS], BF16_T, kind="ExternalInput").ap()
    out_main = nc.dram_tensor("out_main", [N, VS], BF16_T,
                              kind="ExternalOutput").ap()
    out_copy = nc.dram_tensor("out_copy", [N, CVOCAB], F32,
                              kind="ExternalOutput").ap()
    if DEBUG_TAPS:
        den_dbg = nc.dram_tensor("den_dbg", [128, NT], F32,
                                 kind="ExternalOutput").ap()
        omp_dbg = nc.dram_tensor("omp_dbg", [128, NT], F32,
                                 kind="ExternalOutput").ap()

    GROUPS = [list(range(NCORES))]

    with tile.TileContext(nc) as tc, ExitStack() as ctx:
        singles = ctx.enter_context(tc.tile_pool(name="singles", bufs=1))
        dram = ctx.enter_context(tc.tile_pool(name="dram", bufs=1, space="DRAM"))

        # ---- resident inputs (ht tile j and wt chunk j interleaved so the
        # first row tile's matmuls can start ~3us in) ----
        ht_sb = singles.tile([128, NT * 1024], MAIN_T)
        wt_sb = singles.tile([128, DT * VS], MAIN_T)
        # priority order: rows for tiles 0-1, then all weight chunks (they
        # gate the first tiles' matmuls), then the rest
        for j in (0, 1):
            nc.sync.dma_start(out=ht_sb[:, j * 1024:(j + 1) * 1024],
                              in_=ht[:, j * 1024:(j + 1) * 1024])
        for ch in range(NCH):
            blk = DT * CH_O[ch]
            w = DT * CH_W[ch]
            nc.sync.dma_start(out=wt_sb[:, blk:blk + w],
                              in_=wt[:, blk:blk + w])
        for j in range(2, NT):
            nc.sync.dma_start(out=ht_sb[:, j * 1024:(j + 1) * 1024],
                              in_=ht[:, j * 1024:(j + 1) * 1024])
        if USE_FP8:
            htz_sb = singles.tile([128, NT * 1024], BF16_T)
            for j in range(NT):
                nc.sync.dma_start(out=htz_sb[:, j * 1024:(j + 1) * 1024],
                                  in_=htz[:, j * 1024:(j + 1) * 1024])
        else:
            htz_sb = ht_sb
        wc_sb = singles.tile([128, DT], BF16_T)
        nc.sync.dma_start(out=wc_sb, in_=wc)
        mneg13_sb = singles.tile([128, 1], F32)
        nc.gpsimd.dma_start(out=mneg13_sb, in_=mneg13.to_broadcast((128, 1)))
        if use_bgen:
            bg_sb = singles.tile([1, VS], BF16_T)
            nc.sync.dma_start(out=bg_sb, in_=bg)
            ones_sb = singles.tile([1, N], BF16_T)
            nc.vector.memset(ones_sb, 1.0)

        zrow = singles.tile([1, N], F32)        # copy-gate logits, row layout
        zcol = singles.tile([128, NT], F32)     # ... column layout per tile
        ompcol = singles.tile([128, NT], F32)   # 1 - p_copy = sigmoid(-z - bc)
        zbt = singles.tile([TLEN, BATCH], F32)
        pcbt = singles.tile([TLEN, BATCH], F32)  # p_copy = sigmoid(z + bc)

        cps = ctx.enter_context(tc.tile_pool(name="cps", bufs=1))
        ocp = ctx.enter_context(tc.tile_pool(name="ocp", bufs=4))
        expp = ctx.enter_context(tc.tile_pool(name="expp", bufs=9))
        accp = ctx.enter_context(tc.tile_pool(name="accp", bufs=2))
        small = ctx.enter_context(tc.tile_pool(name="small", bufs=4))
        ostp = ctx.enter_context(tc.tile_pool(name="ostp", bufs=3))
        ps_z = ctx.enter_context(tc.tile_pool(name="ps_z", bufs=2,
                                              space="PSUM"))
        ps_cp = ctx.enter_context(
            tc.tile_pool(name="ps_cp", bufs=2, space="PSUM"))
        ps_main = ctx.enter_context(
            tc.tile_pool(name="ps_main", bufs=4, space="PSUM"))

        attn_sb = cps.tile([128, BATCH * TLEN], BF16_T)
        nc.sync.dma_start(out=attn_sb, in_=attn_r)
        sm_sb = cps.tile([128, BATCH * CVOCAB], BF16_T)
        nc.sync.dma_start(out=sm_sb, in_=smap)

        # tiles are grouped; one AllReduce carries a whole group's partial
        # denominators so the CC stream never paces the PE
        GRP = [(0, 4), (4, 4), (8, 4), (12, 4)]
        grp_of = {}
        for g, (s, n) in enumerate(GRP):
            for j in range(s, s + n):
                grp_of[j] = (g, s, n)
        tile_state = {}
        grp_state = {}

        def main_tile_a(j):
            g, s, n = grp_of[j]
            n0 = j * 128
            if j == s:
                grp_state[g] = accp.tile([128, n], F32, name="ccg", tag="ccg",
                                         bufs=3, padded_shape=[128, 4])
            ccg = grp_state[g]
            exp_sb = expp.tile([128, VS], FP8_T, tag="exp")
            acc13 = accp.tile([128, NCH], F32, tag="acc13")
            for ch in range(NCH):
                cw = CH_W[ch]
                c0 = CH_O[ch]
                blk = DT * c0
                psm = ps_main.tile([128, cw], F32, tag="psm",
                                   padded_shape=[128, 512])
                # 512-wide halves: each matmul output stays within one PSUM
                # bank; one Exp activation then covers the whole 2-bank chunk
                halves = ([(0, 512), (512, 512)] if cw >= 1024 else [(0, cw)])
                for h0, hw in halves:
                    hb = blk + DT * h0
                    if USE_FP8:
                        for q in range(DT // 2):
                            lt = ht_sb[:, j * 1024 + q * 256:
                                       j * 1024 + (q + 1) * 256]
                            rt = wt_sb[:, hb + q * 2 * hw:hb + (q + 1) * 2 * hw]
                            nc.tensor.matmul(
                                psm[:, h0:h0 + hw],
                                lhsT=lt.rearrange("p (i r) -> p i r", i=2),
                                rhs=rt.rearrange("p (i v) -> p i v", i=2),
                                start=(q == 0),
                                stop=(q == DT // 2 - 1) and not use_bgen,
                                perf_mode=mybir.MatmulPerfMode.DoubleRow,
                            )
                    else:
                        for d in range(DT):
                            nc.tensor.matmul(
                                psm[:, h0:h0 + hw],
                                lhsT=ht_sb[:, j * 1024 + d * 128:
                                           j * 1024 + d * 128 + 128],
                                rhs=wt_sb[:, hb + d * hw:hb + (d + 1) * hw],
                                start=(d == 0),
                                stop=(d == DT - 1) and not use_bgen,
                            )
                    if use_bgen:
                        nc.tensor.matmul(
                            psm[:, h0:h0 + hw],
                            lhsT=ones_sb[:, n0:n0 + 128],
                            rhs=bg_sb[:, c0 + h0:c0 + h0 + hw],
                            start=False, stop=True,
                        )
                nc.scalar.activation(exp_sb[:, c0:c0 + cw], psm,
                                     mybir.ActivationFunctionType.Exp,
                                     scale=(1.0 / W_SCALE) if USE_FP8 else 1.0,
                                     accum_out=acc13[:, ch:ch + 1])
            # partial denominator = sum(acc13) + 13 * (-mask_count/13), all on
            # the Scalar engine so nothing here can block on a collective
            accd = accp.tile([128, NCH], F32, tag="accd")
            nc.scalar.activation(accd, acc13,
                                 mybir.ActivationFunctionType.Identity,
                                 bias=mneg13_sb, accum_out=ccg[:, j - s:j - s + 1])
            tile_state[j] = exp_sb
            if j == s + n - 1:
                ccin = dram.tile([128, n], F32, name="ccin", tag=f"ccin{n}",
                                 bufs=2)
                nc.scalar.dma_start(out=ccin, in_=ccg)
                ccout = dram.tile([128, n], F32, name="ccout", tag=f"ccout{n}",
                                  bufs=2)
                nc.gpsimd.collective_compute(
                    "AllReduce", mybir.AluOpType.add,
                    replica_groups=GROUPS,
                    ins=[ccin.opt()], outs=[ccout.opt()],
                )
                grp_state[g] = ccout

        def main_tile_b(j):
            g, s, n = grp_of[j]
            n0 = j * 128
            exp_sb = tile_state.pop(j)
            if j == s:
                ccout = grp_state.pop(g)
                den4 = small.tile([128, n], F32, tag="den4",
                                  padded_shape=[128, 4])
                nc.sync.dma_start(out=den4, in_=ccout)
                if DEBUG_TAPS:
                    nc.sync.dma_start(out=den_dbg[:, s:s + n], in_=den4)
                rden4 = small.tile([128, n], F32, tag="rden4",
                                   padded_shape=[128, 4])
                nc.vector.reciprocal(rden4, den4)
                fs4 = small.tile([128, n], F32, tag="fs4",
                                 padded_shape=[128, 4])
                nc.vector.tensor_mul(fs4, rden4, ompcol[:, s:s + n])
                grp_state[(g, "fs")] = fs4
            fs4 = grp_state[(g, "fs")]
            if j == s + n - 1:
                del grp_state[(g, "fs")]
            HW0, HW1 = 3142, VS - 3142
            for h0, hw in ((0, HW0), (HW0, HW1)):
                ost = ostp.tile([128, hw], BF16_T, tag="ost",
                                padded_shape=[128, HW0])
                nc.vector.tensor_scalar_mul(ost, exp_sb[:, h0:h0 + hw],
                                            fs4[:, j - s:j - s + 1])
                nc.sync.dma_start(out=out_main[n0:n0 + 128, h0:h0 + hw],
                                  in_=ost)

        def emit_gate_and_copy():
            # ---- copy-gate z = hidden @ W_copy.T  (M=1 matmuls, ~9us PE),
            # then the copy-attention path.  Emitted after group 0's A phase:
            # ompcol must be written before any B phase reads it (an
            # emission-order read-before-write would be an untracked race).
            emit_z()
            emit_copy()

        def emit_z():
            zs = 1.0
            for j in range(NT):
                zp = ps_z.tile([1, 128], F32, tag="zp")
                for d in range(DT):
                    nc.tensor.matmul(
                        zp,
                        lhsT=wc_sb[:, d:d + 1],
                        rhs=htz_sb[:, j * 1024 + d * 128:
                                   j * 1024 + d * 128 + 128],
                        start=(d == 0), stop=(d == DT - 1),
                    )
                nc.scalar.copy(out=zrow[:, j * 128:(j + 1) * 128], in_=zp)
            zdram = dram.tile([N], F32, tag="zdram")
            nc.sync.dma_start(out=zdram.rearrange("(a n) -> a n", a=1),
                              in_=zrow)
            # row-tile column layout [128, 16] and per-(t,b) layout [64, 32]
            nc.scalar.dma_start(out=zcol,
                                in_=zdram.rearrange("(j p) -> p j", p=128))
            nc.scalar.dma_start(out=zbt,
                                in_=zdram.rearrange("(t b) -> t b", b=BATCH))
            nc.scalar.activation(ompcol, zcol,
                                 mybir.ActivationFunctionType.Sigmoid,
                                 bias=-bc_val, scale=-zs)
            nc.scalar.activation(pcbt, zbt,
                                 mybir.ActivationFunctionType.Sigmoid,
                                 bias=bc_val, scale=zs)
            if DEBUG_TAPS:
                nc.sync.dma_start(out=omp_dbg, in_=ompcol)

        def emit_copy():
            # ---- copy path: per-batch [64t,128s] @ [128s,64c] x p_copy ----
            oc3 = out_copy.rearrange("(t b) c -> t b c", b=BATCH)
            for b in range(BATCH):
                cp = ps_cp.tile([TLEN, CVOCAB], F32, tag="cp")
                nc.tensor.matmul(
                    cp,
                    lhsT=attn_sb[:, b * TLEN:(b + 1) * TLEN],
                    rhs=sm_sb[:, b * CVOCAB:(b + 1) * CVOCAB],
                    start=True, stop=True,
                )
                oc = ocp.tile([TLEN, CVOCAB], F32, tag="oc")
                nc.vector.tensor_scalar_mul(oc, cp, pcbt[:, b:b + 1])
                nc.sync.dma_start(out=oc3[:, b, :], in_=oc)

        # ---- row tiles, grouped ----
        for g, (s, n) in enumerate(GRP):
            for j in range(s, s + n):
                main_tile_a(j)
            if g == 0:
                emit_gate_and_copy()
            for j in range(s, s + n):
                main_tile_b(j)

    nc.compile()
    return nc


def _get_nc(bc_val: float, use_bgen: bool):
    key = (bc_val, use_bgen)
    if key not in _NC_CACHE:
        _NC_CACHE[key] = _build(bc_val, use_bgen)
    return _NC_CACHE[key]


def kernel(hidden, attn, src_map, W_gen, b_gen, W_copy, b_copy):
    global LAST_RESULTS
    hidden = np.asarray(hidden, dtype=np.float32)
    attn = np.asarray(attn, dtype=np.float32)
    src_map = np.asarray(src_map, dtype=np.float32)
    W_gen = np.asarray(W_gen, dtype=np.float32)
    b_gen = np.asarray(b_gen, dtype=np.float32)
    W_copy = np.asarray(W_copy, dtype=np.float32)
    b_copy = np.asarray(b_copy, dtype=np.float32)

    use_bgen = bool(np.any(b_gen))
    bc_val = float(b_copy.reshape(-1)[0])
    nc = _get_nc(bc_val, use_bgen)

    # hidden^T, tile-major: ht[p, j*1024 + d*128 + r] = hidden[j*128+r, d*128+p]
    htz = np.ascontiguousarray(
        hidden.reshape(NT, 128, DT, 128).transpose(3, 0, 2, 1)
    ).reshape(128, NT * 1024).astype(BF16)
    if USE_FP8:
        # d-pair interleaved: ht[p, j*1024 + q*256 + i*128 + r]
        ht = np.ascontiguousarray(
            hidden.reshape(NT, 128, DT // 2, 2, 128).transpose(4, 0, 2, 3, 1)
        ).reshape(128, NT * 1024).astype(FP8)
    else:
        ht = htz

    # padded W with masked rows zeroed (PAD row + vocab padding)
    Wp = np.zeros((VPAD, D), dtype=np.float32)
    Wp[:VOCAB] = W_gen
    Wp[PAD_IDX] = 0.0
    if USE_FP8:
        Wp *= W_SCALE
    if use_bgen:
        bgp = np.zeros((VPAD,), dtype=np.float32)
        bgp[:VOCAB] = b_gen
        bgp[PAD_IDX] = 0.0
        if USE_FP8:
            bgp *= W_SCALE

    # attn rearranged to [s, b, t]
    attn_r = np.ascontiguousarray(
        attn.reshape(TLEN, BATCH, SLEN).transpose(2, 1, 0)
    ).reshape(128, BATCH * TLEN).astype(BF16)
    smap = np.ascontiguousarray(
        src_map.reshape(SLEN, BATCH * CVOCAB)).astype(BF16)
    wc = np.ascontiguousarray(W_copy[0].reshape(DT, 128).T).astype(BF16)

    masked = np.zeros(VPAD, dtype=bool)
    masked[PAD_IDX] = True
    masked[VOCAB:] = True

    in_maps = []
    for c in range(NCORES):
        shard = Wp[c * VS:(c + 1) * VS]           # [VS, D]
        blocks = []
        for ch in range(NCH):
            cw = CH_W[ch]
            arr = shard[CH_O[ch]:CH_O[ch] + cw]    # [cw, D]
            hw = 512 if cw >= 512 else cw
            nh = cw // hw
            if USE_FP8:
                # [p, h, q, i, v]: 512-col half-blocks, d-pairs interleaved
                b = (arr.reshape(nh, hw, DT // 2, 2, 128)
                     .transpose(4, 0, 2, 3, 1))
            else:
                # [p, h, d, v]
                b = arr.reshape(nh, hw, DT, 128).transpose(3, 0, 2, 1)
            blocks.append(
                np.ascontiguousarray(b).reshape(128, DT * cw))
        wt_c = np.concatenate(blocks, axis=1).astype(FP8 if USE_FP8 else BF16)
        mcount = int(masked[c * VS:(c + 1) * VS].sum())
        m = {
            "wt": wt_c,
            "ht": ht,
            "htz": htz,
            "attn_r": attn_r,
            "smap": smap,
            "wc": wc,
            "mneg13": np.array([[-float(mcount) / NCH]], dtype=np.float32),
        }
        if use_bgen:
            m["bg"] = bgp[c * VS:(c + 1) * VS].reshape(1, VS).astype(BF16)
        in_maps.append(m)

    res = run_bass_kernel_spmd(nc, in_maps, core_ids=list(range(NCORES)))
    LAST_RESULTS = res

    out = np.empty((N, VOCAB + CVOCAB), dtype=np.float32)
    for c in range(NCORES):
        lo = c * VS
        hi = min(lo + VS, VOCAB)
        if hi > lo:
            out[:, lo:hi] = res.results[c]["out_main"][:, :hi - lo]
    out[:, PAD_IDX] = 0.0
    out[:, VOCAB:] = res.results[0]["out_copy"]
    return out


if __name__ == "__main__":
    # build-only smoke test
    nc = _get_nc(0.0, False)
    print("build OK:", nc)
